# revision 23
# baseline (speedup 1.0000x reference)
"""MoE router kernel for Trainium2 (8 NeuronCores, data-parallel over tokens).

Computes, for h [N, 512]:
    h_proj = h @ W.T                       [N, 64]
    scores = l2norm(h_proj) @ l2norm(E).T  [N, 64]   (cosine)
    full_gates = softmax(scores / tau)
    top2 -> normalized_topk = softmax(top2 values)
    sparse_gates = scatter(normalized_topk at top2 indices)
Returns (sparse_gates [N,64] f32, topk_indices [N,2] i32, full_gates [N,64] f32).

Device strategy (per core, ns = N/8 tokens):
  - Host folds router params into A = [W.T | (W.T @ e_norm.T)/tau] ([512,128])
    and ships h pre-transposed, split hi/lo into bf16 pairs (3-term product
    hi@Ahi + hi@Alo + lo@Ahi == fp32-class precision, ~5e-6 rel).
  - Per 128-token tile: 12 accumulating bf16 matmuls -> PSUM [128,128]
    holding (h_proj | scores_pre); scores = scores_pre / ||h_proj||.
  - ACT runs ONLY Square (+accum -> ss, no LUT) and Exp (one LUT, never
    reloaded): exp_t = Exp(scores_pre * inv); e1 = Exp(-sdiff) for the
    2-way-softmax sigmoid.
  - inv = rsqrt(ss) via fp32-domain magic seed + 2 Newton iterations on DVE,
    per half-supertile so only 8 PSUM tiles are ever in flight
    (inv scales a token's whole row uniformly -> cannot flip top-2 order).
  - DVE: max8 + find_index8 (top-2 vals+idx in 2 instrs); both one-hot
    (iota==idx)*gate masks built in one [128,2,64] is_equal + one mult pass.
  - GpSimd: fg = exp * (1/Z) and the sparse half-sum (streaming tensor ops
    only -- GpSimd scalar ops are ~1.1us software paths, avoided).
  - The sparse/top-2-weight phase of supertile s is emitted during supertile
    s+1 (1-stage software pipeline) so it overlaps the next matmul burst.
  - Loads: hi|lo packed per supertile (single-stream 8KB runs); all outputs
    in supertile-partition-major layout [nsup, 128, G, *] so every store is
    one address-sequential DMA; host untangles. Top-2 indices are compacted
    to a contiguous tile before the store (a strided SBUF source fragments
    the DMA into 8-byte transfers that head-of-line-block all queues).
"""

import numpy as np

N_TOKENS = 262144
IN_DIM = 512
E = 64          # num experts
DE = 64         # router dim
N_CORES = 8
TPB = 128       # tokens per tile (partition dim)
G = 16          # token tiles per supertile
H = G // 2      # tiles per inv-batch (PSUM capacity = 8 banks)
SUP = G * TPB   # tokens per supertile
KC = IN_DIM // 128  # contraction chunks (4)
MAGICF = float(0x5F3759DF)  # rsqrt seed, applied in fp32 value domain

_CACHE = {}


def _build(ns):
    """Build + compile the per-core Bass program for ns tokens per core."""
    key = ("nc", ns)
    if key in _CACHE:
        return _CACHE[key]

    import concourse.bacc as bacc
    import concourse.tile as tile
    from concourse import mybir

    f32 = mybir.dt.float32
    bf16 = mybir.dt.bfloat16
    u32 = mybir.dt.uint32
    i32 = mybir.dt.int32
    AF = mybir.ActivationFunctionType
    OP = mybir.AluOpType

    nsup = ns // SUP
    ntiles = ns // TPB
    assert nsup * SUP == ns

    nc = bacc.Bacc("TRN2", target_bir_lowering=False, debug=False)
    hP = nc.dram_tensor("hP", [IN_DIM, nsup, 2, SUP], bf16,
                        kind="ExternalInput").ap()
    Ahi = nc.dram_tensor("Ahi", [IN_DIM, 128], bf16, kind="ExternalInput").ap()
    Alo = nc.dram_tensor("Alo", [IN_DIM, 128], bf16, kind="ExternalInput").ap()
    iota = nc.dram_tensor("iota", [TPB, 2, E], f32, kind="ExternalInput").ap()
    spP = nc.dram_tensor("spP", [nsup, TPB, G, E], f32,
                         kind="ExternalOutput").ap()
    idxP = nc.dram_tensor("idxP", [nsup, TPB, G, 2], i32,
                          kind="ExternalOutput").ap()
    fgP = nc.dram_tensor("fgP", [nsup, TPB, G, E], f32,
                         kind="ExternalOutput").ap()

    hP_r = hP.rearrange("(c p) s k n -> p c s k n", p=128)

    with tile.TileContext(nc) as tc:
        with (
            tc.tile_pool(name="const", bufs=1) as const,
            tc.tile_pool(name="hpool", bufs=3) as hpool,
            tc.tile_pool(name="psum", bufs=8, space="PSUM") as psump,
            tc.tile_pool(name="expp", bufs=36) as expp,
            tc.tile_pool(name="outp", bufs=4) as outp,
            tc.tile_pool(name="maskp", bufs=8) as maskp,
            tc.tile_pool(name="stg", bufs=4) as stg,
        ):
            Ah_sb = const.tile([128, KC, 128], bf16, tag="Ah")
            nc.sync.dma_start(Ah_sb, Ahi.rearrange("(c p) m -> p c m", p=128))
            Al_sb = const.tile([128, KC, 128], bf16, tag="Al")
            nc.sync.dma_start(Al_sb, Alo.rearrange("(c p) m -> p c m", p=128))
            iota_sb = const.tile([TPB, 2, E], f32, tag="iota")
            nc.sync.dma_start(iota_sb, iota)

            prev = None

            def _mask_phase(state):
                ps_, mx_, rz_, ix_ = state
                diff_st = stg.tile([TPB, G], f32, tag="diff")
                e1_st = stg.tile([TPB, G], f32, tag="e1")
                g12_st = stg.tile([TPB, G, 2], f32, tag="g12")
                i1f_st = stg.tile([TPB, G, 2], f32, tag="i1f")
                sp_st = outp.tile([TPB, G, E], f32, tag="sp")
                # renormalized top-2: g1 = 1/(1+exp(-(m1-m2)/Z)), g2 = 1-g1
                nc.vector.tensor_tensor(out=diff_st, in0=mx_[:, :, 0],
                                        in1=mx_[:, :, 1], op=OP.subtract)
                nc.vector.tensor_mul(diff_st, diff_st, rz_)
                nc.scalar.activation(out=e1_st, in_=diff_st, func=AF.Exp,
                                     scale=-1.0)
                nc.vector.tensor_scalar_add(e1_st, e1_st, 1.0)
                nc.vector.reciprocal(out=g12_st[:, :, 0], in_=e1_st)
                nc.vector.tensor_scalar(g12_st[:, :, 1], g12_st[:, :, 0],
                                        -1.0, 1.0, op0=OP.mult, op1=OP.add)
                # indices as f32 for the equality masks
                nc.vector.tensor_copy(out=i1f_st, in_=ix_[:, :, 0:2])
                # compact top-2 indices to a contiguous tile: the strided
                # (0:2 of 8) SBUF source would fragment the DMA into 8-byte
                # transfers (~50us/queue of pure overhead)
                ixc_st = stg.tile([TPB, G, 2], u32, tag="ixc")
                nc.vector.tensor_copy(out=ixc_st, in_=ix_[:, :, 0:2])
                for t in range(G):
                    # both one-hot masks in one pass over [128, 2, 64]
                    eq12 = maskp.tile([TPB, 2, E], f32, tag="eq12")
                    nc.vector.tensor_tensor(
                        out=eq12, in0=iota_sb,
                        in1=i1f_st[:, t, :].to_broadcast([TPB, 2, E]),
                        op=OP.is_equal)
                    mul12 = maskp.tile([TPB, 2, E], f32, tag="mul12")
                    nc.vector.tensor_tensor(
                        out=mul12, in0=eq12,
                        in1=g12_st[:, t, :].to_broadcast([TPB, 2, E]),
                        op=OP.mult)
                    nc.gpsimd.tensor_add(sp_st[:, t, :], mul12[:, 0, :],
                                         mul12[:, 1, :])
                nc.sync.dma_start(spP[ps_, :, :, :], sp_st)
                nc.sync.dma_start(idxP[ps_, :, :, :], ixc_st.bitcast(i32))

            for s in range(nsup):
                tok0 = s * SUP
                ht = hpool.tile([128, KC, 2, SUP], bf16, tag="ht")
                nc.sync.dma_start(ht, hP_r[:, :, s, :, :])

                ss_st = stg.tile([TPB, G], f32, tag="ss")
                y_st = stg.tile([TPB, G], f32, tag="y")
                a_st = stg.tile([TPB, G], f32, tag="a")
                z_st = stg.tile([TPB, G], f32, tag="z")
                rz_st = stg.tile([TPB, G], f32, tag="rz")
                mx_st = stg.tile([TPB, G, 8], f32, tag="mx")
                ix_st = stg.tile([TPB, G, 8], u32, tag="ix")
                fg_st = outp.tile([TPB, G, E], f32, tag="fg")

                # 4 token-tiles per PSUM bank tile: 2 full supertiles fit
                # in the 8 banks, so inv/Newton batches the whole supertile
                psb0 = psump.tile([TPB, 4, 128], f32, tag="psb")
                psb1 = psump.tile([TPB, 4, 128], f32, tag="psb")
                psb2 = psump.tile([TPB, 4, 128], f32, tag="psb")
                psb3 = psump.tile([TPB, 4, 128], f32, tag="psb")
                psb = [psb0, psb1, psb2, psb3]
                exps = {}
                for t in range(G):
                    ps = psb[t // 4][:, t % 4, :]
                    sl = slice(t * TPB, (t + 1) * TPB)
                    for c in range(KC):
                        nc.tensor.matmul(ps, lhsT=ht[:, c, 0, sl],
                                         rhs=Ah_sb[:, c, :],
                                         start=(c == 0), stop=False)
                    for c in range(KC):
                        nc.tensor.matmul(ps, lhsT=ht[:, c, 0, sl],
                                         rhs=Al_sb[:, c, :],
                                         start=False, stop=False)
                    for c in range(KC):
                        nc.tensor.matmul(ps, lhsT=ht[:, c, 1, sl],
                                         rhs=Ah_sb[:, c, :],
                                         start=False, stop=(c == KC - 1))
                    # ss[t] = sum(h_proj^2); squares discarded in place
                    nc.scalar.activation(out=ps[:, 0:DE], in_=ps[:, 0:DE],
                                         func=AF.Square,
                                         accum_out=ss_st[:, t:t + 1])

                # inv = rsqrt(ss), whole supertile: fp32-domain magic seed
                # (u32 values are exact enough in fp32) + 2 Newton iterations
                nc.vector.tensor_scalar(y_st.bitcast(u32), ss_st.bitcast(u32),
                                        -0.5, MAGICF,
                                        op0=OP.mult, op1=OP.add)
                for _ in range(2):
                    # y *= 1.5 - 0.5*ss*y^2, fused as 3 ops via stt
                    nc.vector.tensor_mul(a_st, y_st, y_st)
                    nc.vector.scalar_tensor_tensor(
                        a_st, a_st, -0.5, ss_st, op0=OP.mult, op1=OP.mult)
                    nc.vector.scalar_tensor_tensor(
                        y_st, a_st, 1.5, y_st, op0=OP.add, op1=OP.mult)

                for t in range(G):
                    ex = expp.tile([TPB, E], f32)
                    exps[t] = ex
                    nc.scalar.activation(
                        out=ex, in_=psb[t // 4][:, t % 4, DE:128],
                        func=AF.Exp, scale=y_st[:, t:t + 1],
                        accum_out=z_st[:, t:t + 1])

                nc.vector.reciprocal(out=rz_st, in_=z_st)
                if True:
                    for t in range(G):
                        # full gates = exp * (1/Z)  (GpSimd tensor_tensor
                        # with broadcast 1/Z; unloads ACT and DVE)
                        nc.gpsimd.tensor_mul(
                            fg_st[:, t, :], exps[t],
                            rz_st[:, t:t + 1].to_broadcast([TPB, E]))
                        nc.vector.max(out=mx_st[:, t, :], in_=exps[t])
                        nc.vector.max_index(out=ix_st[:, t, :],
                                            in_max=mx_st[:, t, :],
                                            in_values=exps[t])
                nc.sync.dma_start(fgP[s, :, :, :], fg_st)

                # Sparse/top-2 weight phase for the PREVIOUS supertile:
                # deferring it keeps DVE's in-order queue from blocking the
                # next supertile's Newton/exp critical chain (trace showed PE
                # fully idle during the mask backlog otherwise).
                if prev is not None:
                    _mask_phase(prev)
                prev = (s, mx_st, rz_st, ix_st)

            _mask_phase(prev)

    nc.compile()
    _CACHE[key] = nc
    return nc


def _prep_params(W, expert_embeddings, tau):
    import ml_dtypes
    e = expert_embeddings.astype(np.float64)
    e_norm = e / np.maximum(np.linalg.norm(e, axis=1, keepdims=True), 1e-12)
    Wd = W.astype(np.float64)
    A = np.concatenate([Wd.T, (Wd.T @ e_norm.T) / float(tau)], axis=1)
    A = np.ascontiguousarray(A, dtype=np.float32)  # [512, 128]
    A_hi = A.astype(ml_dtypes.bfloat16)
    A_lo = (A - A_hi.astype(np.float32)).astype(ml_dtypes.bfloat16)
    iota = np.ascontiguousarray(
        np.broadcast_to(np.arange(E, dtype=np.float32), (TPB, 2, E)))
    return A_hi, A_lo, iota


def _shard_inputs(h, A_hi, A_lo, iota, ns):
    import ml_dtypes
    nsup = ns // SUP
    in_maps = []
    for c in range(N_CORES):
        shard = np.ascontiguousarray(h[c * ns:(c + 1) * ns].T)  # [512, ns]
        s_hi = shard.astype(ml_dtypes.bfloat16)
        s_lo = (shard - s_hi.astype(np.float32)).astype(ml_dtypes.bfloat16)
        # pack hi|lo per supertile: [512, nsup, 2, SUP] so each supertile
        # load is one single-stream DMA with 8KB contiguous runs
        hp = np.stack([s_hi.reshape(IN_DIM, nsup, SUP),
                       s_lo.reshape(IN_DIM, nsup, SUP)], axis=2)
        in_maps.append({"hP": np.ascontiguousarray(hp), "Ahi": A_hi,
                        "Alo": A_lo, "iota": iota})
    return in_maps


def _untangle(a, ns):
    """[nsup, 128, G, w] device layout -> [ns, w] (token = s*SUP + g*128 + p)."""
    return np.ascontiguousarray(a.transpose(0, 2, 1, 3)).reshape(ns, -1)


def kernel(h, W, expert_embeddings, tau):
    from concourse.bass_utils import run_bass_kernel_spmd

    n = h.shape[0]
    ns = n // N_CORES
    A_hi, A_lo, iota = _prep_params(W, expert_embeddings, tau)
    in_maps = _shard_inputs(h, A_hi, A_lo, iota, ns)

    nc = _build(ns)
    res = run_bass_kernel_spmd(nc, in_maps, core_ids=list(range(N_CORES)))
    sparse = np.concatenate(
        [_untangle(res.results[c]["spP"], ns) for c in range(N_CORES)])
    fg = np.concatenate(
        [_untangle(res.results[c]["fgP"], ns) for c in range(N_CORES)])
    idx = np.concatenate(
        [_untangle(res.results[c]["idxP"], ns) for c in range(N_CORES)])
    return sparse, idx.astype(np.int32), fg


# revision 25
# speedup vs baseline: 1.1200x; 1.1200x over previous
"""MoE router kernel for Trainium2 (8 NeuronCores, data-parallel over tokens).

Computes, for h [N, 512]:
    h_proj = h @ W.T                       [N, 64]
    scores = l2norm(h_proj) @ l2norm(E).T  [N, 64]   (cosine)
    full_gates = softmax(scores / tau)
    top2 -> normalized_topk = softmax(top2 values)
    sparse_gates = scatter(normalized_topk at top2 indices)
Returns (sparse_gates [N,64] f32, topk_indices [N,2] i32, full_gates [N,64] f32).

Device strategy (per core, ns = N/8 tokens):
  - Host folds router params into A = [W.T | (W.T @ e_norm.T)/tau] ([512,128])
    and ships h pre-transposed, split hi/lo into bf16 pairs (3-term product
    hi@Ahi + hi@Alo + lo@Ahi == fp32-class precision, ~5e-6 rel).
  - Per 128-token tile: 12 accumulating bf16 matmuls -> PSUM [128,128]
    holding (h_proj | scores_pre); scores = scores_pre / ||h_proj||.
  - ACT runs ONLY Square (+accum -> ss, no LUT) and Exp (one LUT, never
    reloaded): exp_t = Exp(scores_pre * inv); e1 = Exp(-sdiff) for the
    2-way-softmax sigmoid.
  - inv = rsqrt(ss) via fp32-domain magic seed + 2 Newton iterations on DVE,
    per half-supertile so only 8 PSUM tiles are ever in flight
    (inv scales a token's whole row uniformly -> cannot flip top-2 order).
  - DVE: max8 + find_index8 (top-2 vals+idx in 2 instrs); both one-hot
    (iota==idx)*gate masks built in one [128,2,64] is_equal + one mult pass.
  - GpSimd: fg = exp * (1/Z) and the sparse half-sum (streaming tensor ops
    only -- GpSimd scalar ops are ~1.1us software paths, avoided).
  - The sparse/top-2-weight phase of supertile s is emitted during supertile
    s+1 (1-stage software pipeline) so it overlaps the next matmul burst.
  - Loads: hi|lo packed per supertile (single-stream 8KB runs); all outputs
    in supertile-partition-major layout [nsup, 128, G, *] so every store is
    one address-sequential DMA; host untangles. Top-2 indices are compacted
    to a contiguous tile before the store (a strided SBUF source fragments
    the DMA into 8-byte transfers that head-of-line-block all queues).
"""

import numpy as np

N_TOKENS = 262144
IN_DIM = 512
E = 64          # num experts
DE = 64         # router dim
N_CORES = 8
TPB = 128       # tokens per tile (partition dim)
G = 32          # token tiles per supertile
H = 8           # tiles per inv-batch (PSUM capacity = 8 banks)
SUP = G * TPB   # tokens per supertile
KC = IN_DIM // 128  # contraction chunks (4)
MAGICF = float(0x5F3759DF)  # rsqrt seed, applied in fp32 value domain

_CACHE = {}


def _build(ns):
    """Build + compile the per-core Bass program for ns tokens per core."""
    key = ("nc", ns)
    if key in _CACHE:
        return _CACHE[key]

    import concourse.bacc as bacc
    import concourse.tile as tile
    from concourse import mybir

    f32 = mybir.dt.float32
    bf16 = mybir.dt.bfloat16
    u32 = mybir.dt.uint32
    i32 = mybir.dt.int32
    AF = mybir.ActivationFunctionType
    OP = mybir.AluOpType

    nsup = ns // SUP
    ntiles = ns // TPB
    assert nsup * SUP == ns

    nc = bacc.Bacc("TRN2", target_bir_lowering=False, debug=False)
    hP = nc.dram_tensor("hP", [IN_DIM, nsup, 2, SUP], bf16,
                        kind="ExternalInput").ap()
    Ahi = nc.dram_tensor("Ahi", [IN_DIM, 128], bf16, kind="ExternalInput").ap()
    Alo = nc.dram_tensor("Alo", [IN_DIM, 128], bf16, kind="ExternalInput").ap()
    iota = nc.dram_tensor("iota", [TPB, 2, E], f32, kind="ExternalInput").ap()
    spP = nc.dram_tensor("spP", [nsup, TPB, G, E], f32,
                         kind="ExternalOutput").ap()
    idxP = nc.dram_tensor("idxP", [nsup, TPB, G, 2], i32,
                          kind="ExternalOutput").ap()
    fgP = nc.dram_tensor("fgP", [nsup, TPB, G, E], f32,
                         kind="ExternalOutput").ap()

    hP_r = hP.rearrange("(c p) s k n -> p c s k n", p=128)

    with tile.TileContext(nc) as tc:
        with (
            tc.tile_pool(name="const", bufs=1) as const,
            tc.tile_pool(name="hpool", bufs=2) as hpool,
            tc.tile_pool(name="psum", bufs=8, space="PSUM") as psump,
            tc.tile_pool(name="expp", bufs=28) as expp,
            tc.tile_pool(name="outp", bufs=2) as outp,
            tc.tile_pool(name="maskp", bufs=8) as maskp,
            tc.tile_pool(name="stg", bufs=3) as stg,
        ):
            Ah_sb = const.tile([128, KC, 128], bf16, tag="Ah")
            nc.sync.dma_start(Ah_sb, Ahi.rearrange("(c p) m -> p c m", p=128))
            Al_sb = const.tile([128, KC, 128], bf16, tag="Al")
            nc.sync.dma_start(Al_sb, Alo.rearrange("(c p) m -> p c m", p=128))
            iota_sb = const.tile([TPB, 2, E], f32, tag="iota")
            nc.sync.dma_start(iota_sb, iota)

            prev = None

            def _mask_phase(state):
                ps_, mx_, rz_, ix_ = state
                diff_st = stg.tile([TPB, G], f32, tag="diff")
                e1_st = stg.tile([TPB, G], f32, tag="e1")
                g12_st = stg.tile([TPB, G, 2], f32, tag="g12")
                i1f_st = stg.tile([TPB, G, 2], f32, tag="i1f")
                sp_st = outp.tile([TPB, G, E], f32, tag="sp")
                # renormalized top-2: g1 = 1/(1+exp(-(m1-m2)/Z)), g2 = 1-g1
                nc.vector.tensor_tensor(out=diff_st, in0=mx_[:, :, 0],
                                        in1=mx_[:, :, 1], op=OP.subtract)
                nc.vector.tensor_mul(diff_st, diff_st, rz_)
                nc.scalar.activation(out=e1_st, in_=diff_st, func=AF.Exp,
                                     scale=-1.0)
                nc.vector.tensor_scalar_add(e1_st, e1_st, 1.0)
                nc.vector.reciprocal(out=g12_st[:, :, 0], in_=e1_st)
                nc.vector.tensor_scalar(g12_st[:, :, 1], g12_st[:, :, 0],
                                        -1.0, 1.0, op0=OP.mult, op1=OP.add)
                # indices as f32 for the equality masks
                nc.vector.tensor_copy(out=i1f_st, in_=ix_[:, :, 0:2])
                # compact top-2 indices to a contiguous tile: the strided
                # (0:2 of 8) SBUF source would fragment the DMA into 8-byte
                # transfers (~50us/queue of pure overhead)
                ixc_st = stg.tile([TPB, G, 2], u32, tag="ixc")
                nc.vector.tensor_copy(out=ixc_st, in_=ix_[:, :, 0:2])
                for t in range(G):
                    # both one-hot masks in one pass over [128, 2, 64]
                    eq12 = maskp.tile([TPB, 2, E], f32, tag="eq12")
                    nc.vector.tensor_tensor(
                        out=eq12, in0=iota_sb,
                        in1=i1f_st[:, t, :].to_broadcast([TPB, 2, E]),
                        op=OP.is_equal)
                    mul12 = maskp.tile([TPB, 2, E], f32, tag="mul12")
                    nc.vector.tensor_tensor(
                        out=mul12, in0=eq12,
                        in1=g12_st[:, t, :].to_broadcast([TPB, 2, E]),
                        op=OP.mult)
                    nc.gpsimd.tensor_add(sp_st[:, t, :], mul12[:, 0, :],
                                         mul12[:, 1, :])
                nc.sync.dma_start(spP[ps_, :, :, :], sp_st)
                nc.sync.dma_start(idxP[ps_, :, :, :], ixc_st.bitcast(i32))

            for s in range(nsup):
                tok0 = s * SUP
                ht = hpool.tile([128, KC, 2, SUP], bf16, tag="ht")
                nc.sync.dma_start(ht, hP_r[:, :, s, :, :])

                ss_st = stg.tile([TPB, G], f32, tag="ss")
                y_st = stg.tile([TPB, G], f32, tag="y")
                a_st = stg.tile([TPB, G], f32, tag="a")
                z_st = stg.tile([TPB, G], f32, tag="z")
                rz_st = stg.tile([TPB, G], f32, tag="rz")
                mx_st = stg.tile([TPB, G, 8], f32, tag="mx")
                ix_st = stg.tile([TPB, G, 8], u32, tag="ix")
                fg_st = outp.tile([TPB, G, E], f32, tag="fg")

                psums = {}
                exps = {}
                for half in range(G // H):
                    hsl = slice(half * H, (half + 1) * H)
                    for t in range(half * H, (half + 1) * H):
                        ps = psump.tile([TPB, 128], f32)
                        psums[t] = ps
                        sl = slice(t * TPB, (t + 1) * TPB)
                        for c in range(KC):
                            nc.tensor.matmul(ps, lhsT=ht[:, c, 0, sl],
                                             rhs=Ah_sb[:, c, :],
                                             start=(c == 0), stop=False)
                        for c in range(KC):
                            nc.tensor.matmul(ps, lhsT=ht[:, c, 0, sl],
                                             rhs=Al_sb[:, c, :],
                                             start=False, stop=False)
                        for c in range(KC):
                            nc.tensor.matmul(ps, lhsT=ht[:, c, 1, sl],
                                             rhs=Ah_sb[:, c, :],
                                             start=False, stop=(c == KC - 1))
                        # ss[t] = sum(h_proj^2); squares discarded in place
                        nc.scalar.activation(out=ps[:, 0:DE], in_=ps[:, 0:DE],
                                             func=AF.Square,
                                             accum_out=ss_st[:, t:t + 1])

                    # inv = rsqrt(ss) for this half: fp32-domain magic seed
                    # (u32 values are exact enough in fp32; rounding only
                    # perturbs low seed bits) + 2 Newton iterations.
                    ss_u = ss_st.bitcast(u32)[:, hsl]
                    y_u = y_st.bitcast(u32)[:, hsl]
                    y_f = y_st[:, hsl]
                    a_f = a_st[:, hsl]
                    ss_f = ss_st[:, hsl]
                    nc.vector.tensor_scalar(y_u, ss_u, -0.5, MAGICF,
                                            op0=OP.mult, op1=OP.add)
                    for _ in range(2):
                        # y *= 1.5 - 0.5*ss*y^2, fused as 3 ops via stt
                        nc.vector.tensor_mul(a_f, y_f, y_f)
                        nc.vector.scalar_tensor_tensor(
                            a_f, a_f, -0.5, ss_f, op0=OP.mult, op1=OP.mult)
                        nc.vector.scalar_tensor_tensor(
                            y_f, a_f, 1.5, y_f, op0=OP.add, op1=OP.mult)

                    for t in range(half * H, (half + 1) * H):
                        ex = expp.tile([TPB, E], f32)
                        exps[t] = ex
                        nc.scalar.activation(out=ex, in_=psums[t][:, DE:128],
                                             func=AF.Exp,
                                             scale=y_st[:, t:t + 1],
                                             accum_out=z_st[:, t:t + 1])

                    # rz per half: fg/max of this half don't wait on the
                    # other half's exps
                    nc.vector.reciprocal(out=rz_st[:, hsl], in_=z_st[:, hsl])
                    for t in range(half * H, (half + 1) * H):
                        # full gates = exp * (1/Z)  (GpSimd tensor_tensor
                        # with broadcast 1/Z; unloads ACT and DVE)
                        nc.gpsimd.tensor_mul(
                            fg_st[:, t, :], exps[t],
                            rz_st[:, t:t + 1].to_broadcast([TPB, E]))
                        nc.vector.max(out=mx_st[:, t, :], in_=exps[t])
                        nc.vector.max_index(out=ix_st[:, t, :],
                                            in_max=mx_st[:, t, :],
                                            in_values=exps[t])
                nc.sync.dma_start(fgP[s, :, :, :], fg_st)

                # Sparse/top-2 weight phase for the PREVIOUS supertile:
                # deferring it keeps DVE's in-order queue from blocking the
                # next supertile's Newton/exp critical chain (trace showed PE
                # fully idle during the mask backlog otherwise).
                if prev is not None:
                    _mask_phase(prev)
                prev = (s, mx_st, rz_st, ix_st)

            _mask_phase(prev)

    nc.compile()
    _CACHE[key] = nc
    return nc


def _prep_params(W, expert_embeddings, tau):
    import ml_dtypes
    e = expert_embeddings.astype(np.float64)
    e_norm = e / np.maximum(np.linalg.norm(e, axis=1, keepdims=True), 1e-12)
    Wd = W.astype(np.float64)
    A = np.concatenate([Wd.T, (Wd.T @ e_norm.T) / float(tau)], axis=1)
    A = np.ascontiguousarray(A, dtype=np.float32)  # [512, 128]
    A_hi = A.astype(ml_dtypes.bfloat16)
    A_lo = (A - A_hi.astype(np.float32)).astype(ml_dtypes.bfloat16)
    iota = np.ascontiguousarray(
        np.broadcast_to(np.arange(E, dtype=np.float32), (TPB, 2, E)))
    return A_hi, A_lo, iota


def _shard_inputs(h, A_hi, A_lo, iota, ns):
    import ml_dtypes
    nsup = ns // SUP
    in_maps = []
    for c in range(N_CORES):
        shard = np.ascontiguousarray(h[c * ns:(c + 1) * ns].T)  # [512, ns]
        s_hi = shard.astype(ml_dtypes.bfloat16)
        s_lo = (shard - s_hi.astype(np.float32)).astype(ml_dtypes.bfloat16)
        # pack hi|lo per supertile: [512, nsup, 2, SUP] so each supertile
        # load is one single-stream DMA with 8KB contiguous runs
        hp = np.stack([s_hi.reshape(IN_DIM, nsup, SUP),
                       s_lo.reshape(IN_DIM, nsup, SUP)], axis=2)
        in_maps.append({"hP": np.ascontiguousarray(hp), "Ahi": A_hi,
                        "Alo": A_lo, "iota": iota})
    return in_maps


def _untangle(a, ns):
    """[nsup, 128, G, w] device layout -> [ns, w] (token = s*SUP + g*128 + p)."""
    return np.ascontiguousarray(a.transpose(0, 2, 1, 3)).reshape(ns, -1)


def kernel(h, W, expert_embeddings, tau):
    from concourse.bass_utils import run_bass_kernel_spmd

    n = h.shape[0]
    ns = n // N_CORES
    A_hi, A_lo, iota = _prep_params(W, expert_embeddings, tau)
    in_maps = _shard_inputs(h, A_hi, A_lo, iota, ns)

    nc = _build(ns)
    res = run_bass_kernel_spmd(nc, in_maps, core_ids=list(range(N_CORES)))
    sparse = np.concatenate(
        [_untangle(res.results[c]["spP"], ns) for c in range(N_CORES)])
    fg = np.concatenate(
        [_untangle(res.results[c]["fgP"], ns) for c in range(N_CORES)])
    idx = np.concatenate(
        [_untangle(res.results[c]["idxP"], ns) for c in range(N_CORES)])
    return sparse, idx.astype(np.int32), fg


# revision 26
# speedup vs baseline: 1.1936x; 1.0657x over previous
"""MoE router kernel for Trainium2 (8 NeuronCores, data-parallel over tokens).

Computes, for h [N, 512]:
    h_proj = h @ W.T                       [N, 64]
    scores = l2norm(h_proj) @ l2norm(E).T  [N, 64]   (cosine)
    full_gates = softmax(scores / tau)
    top2 -> normalized_topk = softmax(top2 values)
    sparse_gates = scatter(normalized_topk at top2 indices)
Returns (sparse_gates [N,64] f32, topk_indices [N,2] i32, full_gates [N,64] f32).

Device strategy (per core, ns = N/8 tokens):
  - Host folds router params into A = [W.T | (W.T @ e_norm.T)/tau] ([512,128])
    and ships h pre-transposed, split hi/lo into bf16 pairs (3-term product
    hi@Ahi + hi@Alo + lo@Ahi == fp32-class precision, ~5e-6 rel).
  - Per 128-token tile: 12 accumulating bf16 matmuls -> PSUM [128,128]
    holding (h_proj | scores_pre); scores = scores_pre / ||h_proj||.
  - ACT runs ONLY Square (+accum -> ss, no LUT) and Exp (one LUT, never
    reloaded): exp_t = Exp(scores_pre * inv); e1 = Exp(-sdiff) for the
    2-way-softmax sigmoid.
  - inv = rsqrt(ss) via fp32-domain magic seed + 2 Newton iterations on DVE,
    per half-supertile so only 8 PSUM tiles are ever in flight
    (inv scales a token's whole row uniformly -> cannot flip top-2 order).
  - DVE: max8 + find_index8 (top-2 vals+idx in 2 instrs); both one-hot
    (iota==idx)*gate masks built in one [128,2,64] is_equal + one mult pass.
  - GpSimd: fg = exp * (1/Z) and the sparse half-sum (streaming tensor ops
    only -- GpSimd scalar ops are ~1.1us software paths, avoided).
  - The sparse/top-2-weight phase of supertile s is emitted during supertile
    s+1 (1-stage software pipeline) so it overlaps the next matmul burst.
  - Loads: hi|lo packed per supertile (single-stream 8KB runs); all outputs
    in supertile-partition-major layout [nsup, 128, G, *] so every store is
    one address-sequential DMA; host untangles. Top-2 indices are compacted
    to a contiguous tile before the store (a strided SBUF source fragments
    the DMA into 8-byte transfers that head-of-line-block all queues).
"""

import numpy as np

N_TOKENS = 262144
IN_DIM = 512
E = 64          # num experts
DE = 64         # router dim
N_CORES = 8
TPB = 128       # tokens per tile (partition dim)
G = 16          # token tiles per supertile
H = G // 2      # tiles per inv-batch (PSUM capacity = 8 banks)
SUP = G * TPB   # tokens per supertile
KC = IN_DIM // 128  # contraction chunks (4)
MAGICF = float(0x5F3759DF)  # rsqrt seed, applied in fp32 value domain

_CACHE = {}


def _build(ns):
    """Build + compile the per-core Bass program for ns tokens per core."""
    key = ("nc", ns)
    if key in _CACHE:
        return _CACHE[key]

    import concourse.bacc as bacc
    import concourse.tile as tile
    from concourse import mybir

    f32 = mybir.dt.float32
    bf16 = mybir.dt.bfloat16
    u32 = mybir.dt.uint32
    i32 = mybir.dt.int32
    AF = mybir.ActivationFunctionType
    OP = mybir.AluOpType

    nsup = ns // SUP
    ntiles = ns // TPB
    assert nsup * SUP == ns

    nc = bacc.Bacc("TRN2", target_bir_lowering=False, debug=False)
    hP = nc.dram_tensor("hP", [IN_DIM, nsup, 2, SUP], bf16,
                        kind="ExternalInput").ap()
    Ahi = nc.dram_tensor("Ahi", [IN_DIM, 128], bf16, kind="ExternalInput").ap()
    Alo = nc.dram_tensor("Alo", [IN_DIM, 128], bf16, kind="ExternalInput").ap()
    iota = nc.dram_tensor("iota", [TPB, 2, E], f32, kind="ExternalInput").ap()
    spP = nc.dram_tensor("spP", [nsup, TPB, G, E], f32,
                         kind="ExternalOutput").ap()
    idxP = nc.dram_tensor("idxP", [nsup, TPB, G, 2], i32,
                          kind="ExternalOutput").ap()
    fgP = nc.dram_tensor("fgP", [nsup, TPB, G, E], f32,
                         kind="ExternalOutput").ap()

    hP_r = hP.rearrange("(c p) s k n -> p c s k n", p=128)

    with tile.TileContext(nc) as tc:
        with (
            tc.tile_pool(name="const", bufs=1) as const,
            tc.tile_pool(name="hpool", bufs=3) as hpool,
            tc.tile_pool(name="psum", bufs=8, space="PSUM") as psump,
            tc.tile_pool(name="expp", bufs=36) as expp,
            tc.tile_pool(name="outp", bufs=4) as outp,
            tc.tile_pool(name="maskp", bufs=8) as maskp,
            tc.tile_pool(name="stg", bufs=4) as stg,
        ):
            Ah_sb = const.tile([128, KC, 128], bf16, tag="Ah")
            nc.sync.dma_start(Ah_sb, Ahi.rearrange("(c p) m -> p c m", p=128))
            Al_sb = const.tile([128, KC, 128], bf16, tag="Al")
            nc.sync.dma_start(Al_sb, Alo.rearrange("(c p) m -> p c m", p=128))
            iota_sb = const.tile([TPB, 2, E], f32, tag="iota")
            nc.sync.dma_start(iota_sb, iota)

            prev = None

            def _mask_phase(state):
                ps_, mx_, rz_, ix_ = state
                diff_st = stg.tile([TPB, G], f32, tag="diff")
                e1_st = stg.tile([TPB, G], f32, tag="e1")
                g12_st = stg.tile([TPB, G, 2], f32, tag="g12")
                i1f_st = stg.tile([TPB, G, 2], f32, tag="i1f")
                sp_st = outp.tile([TPB, G, E], f32, tag="sp")
                # renormalized top-2: g1 = 1/(1+exp(-(m1-m2)/Z)), g2 = 1-g1
                nc.vector.tensor_tensor(out=diff_st, in0=mx_[:, :, 0],
                                        in1=mx_[:, :, 1], op=OP.subtract)
                nc.vector.tensor_mul(diff_st, diff_st, rz_)
                nc.scalar.activation(out=e1_st, in_=diff_st, func=AF.Exp,
                                     scale=-1.0)
                nc.vector.tensor_scalar_add(e1_st, e1_st, 1.0)
                nc.vector.reciprocal(out=g12_st[:, :, 0], in_=e1_st)
                nc.vector.tensor_scalar(g12_st[:, :, 1], g12_st[:, :, 0],
                                        -1.0, 1.0, op0=OP.mult, op1=OP.add)
                # indices as f32 for the equality masks
                nc.vector.tensor_copy(out=i1f_st, in_=ix_[:, :, 0:2])
                # compact top-2 indices to a contiguous tile: the strided
                # (0:2 of 8) SBUF source would fragment the DMA into 8-byte
                # transfers (~50us/queue of pure overhead)
                ixc_st = stg.tile([TPB, G, 2], u32, tag="ixc")
                nc.vector.tensor_copy(out=ixc_st, in_=ix_[:, :, 0:2])
                for t in range(G):
                    # both one-hot masks in one pass over [128, 2, 64]
                    eq12 = maskp.tile([TPB, 2, E], f32, tag="eq12")
                    nc.vector.tensor_tensor(
                        out=eq12, in0=iota_sb,
                        in1=i1f_st[:, t, :].to_broadcast([TPB, 2, E]),
                        op=OP.is_equal)
                    mul12 = maskp.tile([TPB, 2, E], f32, tag="mul12")
                    nc.vector.tensor_tensor(
                        out=mul12, in0=eq12,
                        in1=g12_st[:, t, :].to_broadcast([TPB, 2, E]),
                        op=OP.mult)
                    nc.gpsimd.tensor_add(sp_st[:, t, :], mul12[:, 0, :],
                                         mul12[:, 1, :])
                nc.sync.dma_start(spP[ps_, :, :, :], sp_st)
                nc.sync.dma_start(idxP[ps_, :, :, :], ixc_st.bitcast(i32))

            for s in range(nsup):
                tok0 = s * SUP
                ht = hpool.tile([128, KC, 2, SUP], bf16, tag="ht")
                nc.sync.dma_start(ht, hP_r[:, :, s, :, :])

                ss_st = stg.tile([TPB, G], f32, tag="ss")
                y_st = stg.tile([TPB, G], f32, tag="y")
                a_st = stg.tile([TPB, G], f32, tag="a")
                z_st = stg.tile([TPB, G], f32, tag="z")
                rz_st = stg.tile([TPB, G], f32, tag="rz")
                mx_st = stg.tile([TPB, G, 8], f32, tag="mx")
                ix_st = stg.tile([TPB, G, 8], u32, tag="ix")
                fg_st = outp.tile([TPB, G, E], f32, tag="fg")

                psums = {}
                exps = {}
                for half in range(2):
                    hsl = slice(half * H, (half + 1) * H)
                    for t in range(half * H, (half + 1) * H):
                        ps = psump.tile([TPB, 128], f32)
                        psums[t] = ps
                        sl = slice(t * TPB, (t + 1) * TPB)
                        for c in range(KC):
                            nc.tensor.matmul(ps, lhsT=ht[:, c, 0, sl],
                                             rhs=Ah_sb[:, c, :],
                                             start=(c == 0), stop=False)
                        for c in range(KC):
                            nc.tensor.matmul(ps, lhsT=ht[:, c, 0, sl],
                                             rhs=Al_sb[:, c, :],
                                             start=False, stop=False)
                        for c in range(KC):
                            nc.tensor.matmul(ps, lhsT=ht[:, c, 1, sl],
                                             rhs=Ah_sb[:, c, :],
                                             start=False, stop=(c == KC - 1))
                        # ss[t] = sum(h_proj^2); squares discarded in place
                        nc.scalar.activation(out=ps[:, 0:DE], in_=ps[:, 0:DE],
                                             func=AF.Square,
                                             accum_out=ss_st[:, t:t + 1])

                    # inv = rsqrt(ss) for this half: fp32-domain magic seed
                    # (u32 values are exact enough in fp32; rounding only
                    # perturbs low seed bits) + 2 Newton iterations.
                    ss_u = ss_st.bitcast(u32)[:, hsl]
                    y_u = y_st.bitcast(u32)[:, hsl]
                    y_f = y_st[:, hsl]
                    a_f = a_st[:, hsl]
                    ss_f = ss_st[:, hsl]
                    nc.vector.tensor_scalar(y_u, ss_u, -0.5, MAGICF,
                                            op0=OP.mult, op1=OP.add)
                    for _ in range(2):
                        # y *= 1.5 - 0.5*ss*y^2, fused as 3 ops via stt
                        nc.vector.tensor_mul(a_f, y_f, y_f)
                        nc.vector.scalar_tensor_tensor(
                            a_f, a_f, -0.5, ss_f, op0=OP.mult, op1=OP.mult)
                        nc.vector.scalar_tensor_tensor(
                            y_f, a_f, 1.5, y_f, op0=OP.add, op1=OP.mult)

                    for t in range(half * H, (half + 1) * H):
                        ex = expp.tile([TPB, E], f32)
                        exps[t] = ex
                        nc.scalar.activation(out=ex, in_=psums[t][:, DE:128],
                                             func=AF.Exp,
                                             scale=y_st[:, t:t + 1],
                                             accum_out=z_st[:, t:t + 1])

                    # rz per half: fg/max of this half don't wait on the
                    # other half's exps
                    nc.vector.reciprocal(out=rz_st[:, hsl], in_=z_st[:, hsl])
                    for t in range(half * H, (half + 1) * H):
                        # full gates = exp * (1/Z)  (GpSimd tensor_tensor
                        # with broadcast 1/Z; unloads ACT and DVE)
                        nc.gpsimd.tensor_mul(
                            fg_st[:, t, :], exps[t],
                            rz_st[:, t:t + 1].to_broadcast([TPB, E]))
                        nc.vector.max(out=mx_st[:, t, :], in_=exps[t])
                        nc.vector.max_index(out=ix_st[:, t, :],
                                            in_max=mx_st[:, t, :],
                                            in_values=exps[t])
                nc.sync.dma_start(fgP[s, :, :, :], fg_st)

                # Sparse/top-2 weight phase for the PREVIOUS supertile:
                # deferring it keeps DVE's in-order queue from blocking the
                # next supertile's Newton/exp critical chain (trace showed PE
                # fully idle during the mask backlog otherwise).
                if prev is not None:
                    _mask_phase(prev)
                prev = (s, mx_st, rz_st, ix_st)

            _mask_phase(prev)

    nc.compile()
    _CACHE[key] = nc
    return nc


def _prep_params(W, expert_embeddings, tau):
    import ml_dtypes
    e = expert_embeddings.astype(np.float64)
    e_norm = e / np.maximum(np.linalg.norm(e, axis=1, keepdims=True), 1e-12)
    Wd = W.astype(np.float64)
    A = np.concatenate([Wd.T, (Wd.T @ e_norm.T) / float(tau)], axis=1)
    A = np.ascontiguousarray(A, dtype=np.float32)  # [512, 128]
    A_hi = A.astype(ml_dtypes.bfloat16)
    A_lo = (A - A_hi.astype(np.float32)).astype(ml_dtypes.bfloat16)
    iota = np.ascontiguousarray(
        np.broadcast_to(np.arange(E, dtype=np.float32), (TPB, 2, E)))
    return A_hi, A_lo, iota


def _shard_inputs(h, A_hi, A_lo, iota, ns):
    import ml_dtypes
    nsup = ns // SUP
    in_maps = []
    for c in range(N_CORES):
        shard = np.ascontiguousarray(h[c * ns:(c + 1) * ns].T)  # [512, ns]
        s_hi = shard.astype(ml_dtypes.bfloat16)
        s_lo = (shard - s_hi.astype(np.float32)).astype(ml_dtypes.bfloat16)
        # pack hi|lo per supertile: [512, nsup, 2, SUP] so each supertile
        # load is one single-stream DMA with 8KB contiguous runs
        hp = np.stack([s_hi.reshape(IN_DIM, nsup, SUP),
                       s_lo.reshape(IN_DIM, nsup, SUP)], axis=2)
        in_maps.append({"hP": np.ascontiguousarray(hp), "Ahi": A_hi,
                        "Alo": A_lo, "iota": iota})
    return in_maps


def _untangle(a, ns):
    """[nsup, 128, G, w] device layout -> [ns, w] (token = s*SUP + g*128 + p)."""
    return np.ascontiguousarray(a.transpose(0, 2, 1, 3)).reshape(ns, -1)


def kernel(h, W, expert_embeddings, tau):
    from concourse.bass_utils import run_bass_kernel_spmd

    n = h.shape[0]
    ns = n // N_CORES
    A_hi, A_lo, iota = _prep_params(W, expert_embeddings, tau)
    in_maps = _shard_inputs(h, A_hi, A_lo, iota, ns)

    nc = _build(ns)
    res = run_bass_kernel_spmd(nc, in_maps, core_ids=list(range(N_CORES)))
    sparse = np.concatenate(
        [_untangle(res.results[c]["spP"], ns) for c in range(N_CORES)])
    fg = np.concatenate(
        [_untangle(res.results[c]["fgP"], ns) for c in range(N_CORES)])
    idx = np.concatenate(
        [_untangle(res.results[c]["idxP"], ns) for c in range(N_CORES)])
    return sparse, idx.astype(np.int32), fg


# revision 27
# speedup vs baseline: 1.2033x; 1.0081x over previous
"""MoE router kernel for Trainium2 (8 NeuronCores, data-parallel over tokens).

Computes, for h [N, 512]:
    h_proj = h @ W.T                       [N, 64]
    scores = l2norm(h_proj) @ l2norm(E).T  [N, 64]   (cosine)
    full_gates = softmax(scores / tau)
    top2 -> normalized_topk = softmax(top2 values)
    sparse_gates = scatter(normalized_topk at top2 indices)
Returns (sparse_gates [N,64] f32, topk_indices [N,2] i32, full_gates [N,64] f32).

Device strategy (per core, ns = N/8 tokens):
  - Host folds router params into A = [W.T | (W.T @ e_norm.T)/tau] ([512,128])
    and ships h pre-transposed, split hi/lo into bf16 pairs (3-term product
    hi@Ahi + hi@Alo + lo@Ahi == fp32-class precision, ~5e-6 rel).
  - Per 128-token tile: 12 accumulating bf16 matmuls -> PSUM [128,128]
    holding (h_proj | scores_pre); scores = scores_pre / ||h_proj||.
  - ACT runs ONLY Square (+accum -> ss, no LUT) and Exp (one LUT, never
    reloaded): exp_t = Exp(scores_pre * inv); e1 = Exp(-sdiff) for the
    2-way-softmax sigmoid.
  - inv = rsqrt(ss) via fp32-domain magic seed + 2 Newton iterations on DVE,
    per half-supertile so only 8 PSUM tiles are ever in flight
    (inv scales a token's whole row uniformly -> cannot flip top-2 order).
  - DVE: max8 + find_index8 (top-2 vals+idx in 2 instrs); both one-hot
    (iota==idx)*gate masks built in one [128,2,64] is_equal + one mult pass.
  - GpSimd: fg = exp * (1/Z) and the sparse half-sum (streaming tensor ops
    only -- GpSimd scalar ops are ~1.1us software paths, avoided).
  - The sparse/top-2-weight phase of supertile s is emitted during supertile
    s+1 (1-stage software pipeline) so it overlaps the next matmul burst.
  - Loads: hi|lo packed per supertile (single-stream 8KB runs); all outputs
    in supertile-partition-major layout [nsup, 128, G, *] so every store is
    one address-sequential DMA; host untangles. Top-2 indices are compacted
    to a contiguous tile before the store (a strided SBUF source fragments
    the DMA into 8-byte transfers that head-of-line-block all queues).
"""

import numpy as np

N_TOKENS = 262144
IN_DIM = 512
E = 64          # num experts
DE = 64         # router dim
N_CORES = 8
TPB = 128       # tokens per tile (partition dim)
G = 16          # token tiles per supertile
H = G // 2      # tiles per inv-batch (PSUM capacity = 8 banks)
SUP = G * TPB   # tokens per supertile
KC = IN_DIM // 128  # contraction chunks (4)
MAGICF = float(0x5F3759DF)  # rsqrt seed, applied in fp32 value domain

_CACHE = {}


def _build(ns):
    """Build + compile the per-core Bass program for ns tokens per core."""
    key = ("nc", ns)
    if key in _CACHE:
        return _CACHE[key]

    import concourse.bacc as bacc
    import concourse.tile as tile
    from concourse import mybir

    f32 = mybir.dt.float32
    bf16 = mybir.dt.bfloat16
    u32 = mybir.dt.uint32
    i32 = mybir.dt.int32
    AF = mybir.ActivationFunctionType
    OP = mybir.AluOpType

    nsup = ns // SUP
    ntiles = ns // TPB
    assert nsup * SUP == ns

    nc = bacc.Bacc("TRN2", target_bir_lowering=False, debug=False)
    hP = nc.dram_tensor("hP", [IN_DIM, nsup, 2, SUP], bf16,
                        kind="ExternalInput").ap()
    Ahi = nc.dram_tensor("Ahi", [IN_DIM, 128], bf16, kind="ExternalInput").ap()
    Alo = nc.dram_tensor("Alo", [IN_DIM, 128], bf16, kind="ExternalInput").ap()
    iota = nc.dram_tensor("iota", [TPB, 2, E], f32, kind="ExternalInput").ap()
    spP = nc.dram_tensor("spP", [nsup, TPB, G, E], f32,
                         kind="ExternalOutput").ap()
    idxP = nc.dram_tensor("idxP", [nsup, TPB, G, 2], i32,
                          kind="ExternalOutput").ap()
    fgP = nc.dram_tensor("fgP", [nsup, TPB, G, E], f32,
                         kind="ExternalOutput").ap()

    hP_r = hP.rearrange("(c p) s k n -> p c s k n", p=128)

    with tile.TileContext(nc) as tc:
        with (
            tc.tile_pool(name="const", bufs=1) as const,
            tc.tile_pool(name="hpool", bufs=3) as hpool,
            tc.tile_pool(name="psum", bufs=8, space="PSUM") as psump,
            tc.tile_pool(name="expp", bufs=36) as expp,
            tc.tile_pool(name="outp", bufs=4) as outp,
            tc.tile_pool(name="maskp", bufs=12) as maskp,
            tc.tile_pool(name="stg", bufs=5) as stg,
        ):
            Ah_sb = const.tile([128, KC, 128], bf16, tag="Ah")
            nc.sync.dma_start(Ah_sb, Ahi.rearrange("(c p) m -> p c m", p=128))
            Al_sb = const.tile([128, KC, 128], bf16, tag="Al")
            nc.sync.dma_start(Al_sb, Alo.rearrange("(c p) m -> p c m", p=128))
            iota_sb = const.tile([TPB, 2, E], f32, tag="iota")
            nc.sync.dma_start(iota_sb, iota)

            prev = None

            def _mask_phase(state):
                ps_, mx_, rz_, ix_ = state
                diff_st = stg.tile([TPB, G], f32, tag="diff")
                e1_st = stg.tile([TPB, G], f32, tag="e1")
                g12_st = stg.tile([TPB, G, 2], f32, tag="g12")
                i1f_st = stg.tile([TPB, G, 2], f32, tag="i1f")
                sp_st = outp.tile([TPB, G, E], f32, tag="sp")
                # renormalized top-2: g1 = 1/(1+exp(-(m1-m2)/Z)), g2 = 1-g1
                nc.vector.tensor_tensor(out=diff_st, in0=mx_[:, :, 0],
                                        in1=mx_[:, :, 1], op=OP.subtract)
                nc.vector.tensor_mul(diff_st, diff_st, rz_)
                nc.scalar.activation(out=e1_st, in_=diff_st, func=AF.Exp,
                                     scale=-1.0)
                nc.vector.tensor_scalar_add(e1_st, e1_st, 1.0)
                nc.vector.reciprocal(out=g12_st[:, :, 0], in_=e1_st)
                nc.vector.tensor_scalar(g12_st[:, :, 1], g12_st[:, :, 0],
                                        -1.0, 1.0, op0=OP.mult, op1=OP.add)
                # indices as f32 for the equality masks
                nc.vector.tensor_copy(out=i1f_st, in_=ix_[:, :, 0:2])
                # compact top-2 indices to a contiguous tile: the strided
                # (0:2 of 8) SBUF source would fragment the DMA into 8-byte
                # transfers (~50us/queue of pure overhead)
                ixc_st = stg.tile([TPB, G, 2], u32, tag="ixc")
                nc.vector.tensor_copy(out=ixc_st, in_=ix_[:, :, 0:2])
                for t in range(G):
                    # both one-hot masks in one pass over [128, 2, 64]
                    eq12 = maskp.tile([TPB, 2, E], f32, tag="eq12")
                    nc.vector.tensor_tensor(
                        out=eq12, in0=iota_sb,
                        in1=i1f_st[:, t, :].to_broadcast([TPB, 2, E]),
                        op=OP.is_equal)
                    mul12 = maskp.tile([TPB, 2, E], f32, tag="mul12")
                    nc.vector.tensor_tensor(
                        out=mul12, in0=eq12,
                        in1=g12_st[:, t, :].to_broadcast([TPB, 2, E]),
                        op=OP.mult)
                    nc.gpsimd.tensor_add(sp_st[:, t, :], mul12[:, 0, :],
                                         mul12[:, 1, :])
                nc.sync.dma_start(spP[ps_, :, :, :], sp_st)
                nc.sync.dma_start(idxP[ps_, :, :, :], ixc_st.bitcast(i32))

            for s in range(nsup):
                tok0 = s * SUP
                ht = hpool.tile([128, KC, 2, SUP], bf16, tag="ht")
                nc.sync.dma_start(ht, hP_r[:, :, s, :, :])

                ss_st = stg.tile([TPB, G], f32, tag="ss")
                y_st = stg.tile([TPB, G], f32, tag="y")
                a_st = stg.tile([TPB, G], f32, tag="a")
                z_st = stg.tile([TPB, G], f32, tag="z")
                rz_st = stg.tile([TPB, G], f32, tag="rz")
                mx_st = stg.tile([TPB, G, 8], f32, tag="mx")
                ix_st = stg.tile([TPB, G, 8], u32, tag="ix")
                fg_st = outp.tile([TPB, G, E], f32, tag="fg")

                psums = {}
                exps = {}
                for half in range(2):
                    hsl = slice(half * H, (half + 1) * H)
                    for t in range(half * H, (half + 1) * H):
                        ps = psump.tile([TPB, 128], f32)
                        psums[t] = ps
                        sl = slice(t * TPB, (t + 1) * TPB)
                        for c in range(KC):
                            nc.tensor.matmul(ps, lhsT=ht[:, c, 0, sl],
                                             rhs=Ah_sb[:, c, :],
                                             start=(c == 0), stop=False)
                        for c in range(KC):
                            nc.tensor.matmul(ps, lhsT=ht[:, c, 0, sl],
                                             rhs=Al_sb[:, c, :],
                                             start=False, stop=False)
                        for c in range(KC):
                            nc.tensor.matmul(ps, lhsT=ht[:, c, 1, sl],
                                             rhs=Ah_sb[:, c, :],
                                             start=False, stop=(c == KC - 1))
                        # ss[t] = sum(h_proj^2); squares discarded in place
                        nc.scalar.activation(out=ps[:, 0:DE], in_=ps[:, 0:DE],
                                             func=AF.Square,
                                             accum_out=ss_st[:, t:t + 1])

                    # inv = rsqrt(ss) for this half: fp32-domain magic seed
                    # (u32 values are exact enough in fp32; rounding only
                    # perturbs low seed bits) + 2 Newton iterations.
                    ss_u = ss_st.bitcast(u32)[:, hsl]
                    y_u = y_st.bitcast(u32)[:, hsl]
                    y_f = y_st[:, hsl]
                    a_f = a_st[:, hsl]
                    ss_f = ss_st[:, hsl]
                    nc.vector.tensor_scalar(y_u, ss_u, -0.5, MAGICF,
                                            op0=OP.mult, op1=OP.add)
                    for _ in range(2):
                        # y *= 1.5 - 0.5*ss*y^2, fused as 3 ops via stt
                        nc.vector.tensor_mul(a_f, y_f, y_f)
                        nc.vector.scalar_tensor_tensor(
                            a_f, a_f, -0.5, ss_f, op0=OP.mult, op1=OP.mult)
                        nc.vector.scalar_tensor_tensor(
                            y_f, a_f, 1.5, y_f, op0=OP.add, op1=OP.mult)

                    for t in range(half * H, (half + 1) * H):
                        ex = expp.tile([TPB, E], f32)
                        exps[t] = ex
                        nc.scalar.activation(out=ex, in_=psums[t][:, DE:128],
                                             func=AF.Exp,
                                             scale=y_st[:, t:t + 1],
                                             accum_out=z_st[:, t:t + 1])

                    # rz per half: fg/max of this half don't wait on the
                    # other half's exps
                    nc.vector.reciprocal(out=rz_st[:, hsl], in_=z_st[:, hsl])
                    for t in range(half * H, (half + 1) * H):
                        # full gates = exp * (1/Z)  (GpSimd tensor_tensor
                        # with broadcast 1/Z; unloads ACT and DVE)
                        nc.gpsimd.tensor_mul(
                            fg_st[:, t, :], exps[t],
                            rz_st[:, t:t + 1].to_broadcast([TPB, E]))
                        nc.vector.max(out=mx_st[:, t, :], in_=exps[t])
                        nc.vector.max_index(out=ix_st[:, t, :],
                                            in_max=mx_st[:, t, :],
                                            in_values=exps[t])
                nc.sync.dma_start(fgP[s, :, :, :], fg_st)

                # Sparse/top-2 weight phase for the PREVIOUS supertile:
                # deferring it keeps DVE's in-order queue from blocking the
                # next supertile's Newton/exp critical chain (trace showed PE
                # fully idle during the mask backlog otherwise).
                if prev is not None:
                    _mask_phase(prev)
                prev = (s, mx_st, rz_st, ix_st)

            _mask_phase(prev)

    nc.compile()
    _CACHE[key] = nc
    return nc


def _prep_params(W, expert_embeddings, tau):
    import ml_dtypes
    e = expert_embeddings.astype(np.float64)
    e_norm = e / np.maximum(np.linalg.norm(e, axis=1, keepdims=True), 1e-12)
    Wd = W.astype(np.float64)
    A = np.concatenate([Wd.T, (Wd.T @ e_norm.T) / float(tau)], axis=1)
    A = np.ascontiguousarray(A, dtype=np.float32)  # [512, 128]
    A_hi = A.astype(ml_dtypes.bfloat16)
    A_lo = (A - A_hi.astype(np.float32)).astype(ml_dtypes.bfloat16)
    iota = np.ascontiguousarray(
        np.broadcast_to(np.arange(E, dtype=np.float32), (TPB, 2, E)))
    return A_hi, A_lo, iota


def _shard_inputs(h, A_hi, A_lo, iota, ns):
    import ml_dtypes
    nsup = ns // SUP
    in_maps = []
    for c in range(N_CORES):
        shard = np.ascontiguousarray(h[c * ns:(c + 1) * ns].T)  # [512, ns]
        s_hi = shard.astype(ml_dtypes.bfloat16)
        s_lo = (shard - s_hi.astype(np.float32)).astype(ml_dtypes.bfloat16)
        # pack hi|lo per supertile: [512, nsup, 2, SUP] so each supertile
        # load is one single-stream DMA with 8KB contiguous runs
        hp = np.stack([s_hi.reshape(IN_DIM, nsup, SUP),
                       s_lo.reshape(IN_DIM, nsup, SUP)], axis=2)
        in_maps.append({"hP": np.ascontiguousarray(hp), "Ahi": A_hi,
                        "Alo": A_lo, "iota": iota})
    return in_maps


def _untangle(a, ns):
    """[nsup, 128, G, w] device layout -> [ns, w] (token = s*SUP + g*128 + p)."""
    return np.ascontiguousarray(a.transpose(0, 2, 1, 3)).reshape(ns, -1)


def kernel(h, W, expert_embeddings, tau):
    from concourse.bass_utils import run_bass_kernel_spmd

    n = h.shape[0]
    ns = n // N_CORES
    A_hi, A_lo, iota = _prep_params(W, expert_embeddings, tau)
    in_maps = _shard_inputs(h, A_hi, A_lo, iota, ns)

    nc = _build(ns)
    res = run_bass_kernel_spmd(nc, in_maps, core_ids=list(range(N_CORES)))
    sparse = np.concatenate(
        [_untangle(res.results[c]["spP"], ns) for c in range(N_CORES)])
    fg = np.concatenate(
        [_untangle(res.results[c]["fgP"], ns) for c in range(N_CORES)])
    idx = np.concatenate(
        [_untangle(res.results[c]["idxP"], ns) for c in range(N_CORES)])
    return sparse, idx.astype(np.int32), fg
